# revision 27
# baseline (speedup 1.0000x reference)
"""Trainium2 Bass kernel for the attention-mechanism problem.

Math (reference):
    W_s, W_e = W[:SD], W[SD:]
    score[b]    = state_tm1[b] @ W_s + b_bias                 # [B]
    logits[n,b] = emb[n,b] @ W_e + score[b]                   # [N,B]
    alpha       = softmax(logits, axis=0)                     # over N
    out[b,e]    = sum_n alpha[n,b] * emb[n,b,e]               # [B,ED]

score[b] is constant over the softmax axis N, so it cancels: the kernel
never touches state_tm1 / W_s / b_bias at all.

Strategy: data-parallel over B across 8 cores (B_local = 8 per core).
Per-core shard is laid out b-major on the host: row r = b*N + n, so each
[128, ED] SBUF tile holds 128 consecutive n for ONE b.  Single pass:
    l[r]  = emb[r] . W_e          (rowdot -> per-partition scalar)
    p     = exp(l)                (ScalarE, batched per DMA group)
    acc[b]+= p_tile.T @ tile      (TensorE, PSUM accumulation)
    Z[b]  = sum p                 (free-axis reduce + tiny matmul)
    out   = acc / Z

PE constraint: PSUM matmul outputs must sit at base partition 0, so the
weighted sum needs an [128, BL] lhsT whose only nonzero column is b(t).
Rather than building it per tile (the old mask*p ScalarE op), exp writes
p into the k=7 block of a zero-initialized [128, 15*128] strip `pz`;
the strided view pz[:, (896+t-128b)+128k] for k=0..7 then hits the data
column exactly at position b and zeros elsewhere — a free lhsT.

The rowdot is the dominant elementwise cost (512 MACs/row).  Within
EVERY group the tiles are split across three engine lanes (keeping each
lane's progress locked to the DMA stream, so no lane can fall behind
and stall the in-order PE queue):
    D: VectorE scalar_tensor_tensor w/ accum  (1x mode, ~593ns/tile)
    A: VectorE tensor_mul (2x, ~327ns) + ScalarE Copy w/ accum_out
    G: GpSimd tensor_mul (~1.1us, software path; the backend rejects
       STT/accum on Pool) + VectorE tensor_scalar accum (4x, ~253ns)
Each lane gets its own lg tile (single writer) and its own exp call.
With the per-tile mask-multiply gone and the rowdot spread out, every
compute engine sits below the ~45-50us HBM streaming floor for the
16MB fp16 shard, making the kernel DMA-bound.
"""

import os

import numpy as np

N, B, SD, ED = 2048, 64, 512, 512
NCORES = 8
BL = B // NCORES  # 8 batch entries per core
P = 128  # SBUF partitions
NT = (N * BL) // P  # 128 tiles of [128, ED] per core
TPB = N // P  # 16 tiles per batch entry

# Storage/compute precision for the embeddings (the 256MB streaming input):
#   "float16"  (default): half the HBM traffic, rel err ~3e-4 vs the reference
#   "bfloat16": like float16 but rel err ~2e-3
#   "float32":  exact-ish, but 2x the DMA traffic and 4x slower matmul
COMPUTE_DTYPE = os.environ.get("ATTN_KERNEL_DTYPE", "float16")

# Best-known build configuration per dtype.
_BUILD_CFG = {
    "float16": dict(grp=8, emb_bufs=12, taper=True, fd=0.31, fa=0.38),
    "bfloat16": dict(grp=8, emb_bufs=12, taper=True, fd=0.31, fa=0.38),
    "float32": dict(grp=4, emb_bufs=10, taper=True, fd=0.31, fa=0.38),
}

_cache: dict = {}
last_result = None  # BassKernelResults of the most recent run (for profiling)


def _group_sizes(nt: int, grp: int, taper: bool):
    if taper:
        head = [1, 1, 2, 4]
        tail = [4, 2, 1, 1]
        mid_total = nt - sum(head) - sum(tail)
        assert mid_total % grp == 0
        groups = head + [grp] * (mid_total // grp) + tail
    else:
        assert nt % grp == 0
        groups = [grp] * (nt // grp)
    assert sum(groups) == nt
    return groups


def _lane_splits(groups, fd: float, fa: float):
    """Per-group (d, a, g) tile counts hitting the global fd/fa fractions via
    stateful rounding.  Tiny groups go entirely to the D lane (single-op
    chain -> fastest pipeline start/drain)."""
    splits = []
    done = used_d = used_a = 0
    for s in groups:
        done += s
        if s <= 2:
            d, a = s, 0
        else:
            d = max(0, int(round(fd * done)) - used_d)
            a = max(0, int(round((fd + fa) * done)) - used_a - d - used_d + used_d)
            a = max(0, int(round((fd + fa) * done)) - used_d - used_a - d)
            d = min(d, s)
            a = min(a, s - d)
        used_d += d
        used_a += a
        splits.append((d, a, s - d - a))
    return splits


def _build(
    dt_name: str,
    n: int = N,
    grp: int = 8,
    reps: int = 1,
    emb_bufs: int = 12,
    tmp_bufs: int = 3,
    taper: bool = True,
    fd: float = 0.31,
    fa: float = 0.31,
):
    """reps>1 wraps the whole kernel in a device-side For_i loop — used only
    for timing (one RPC amortizes `reps` kernel executions)."""
    import concourse.mybir as mybir
    import concourse.tile as tile
    from concourse import bacc
    from contextlib import nullcontext

    f32 = mybir.dt.float32
    dt = getattr(mybir.dt, dt_name)
    nt = (n * BL) // P
    groups = _group_sizes(nt, grp, taper)
    splits = _lane_splits(groups, fd, fa)

    # pz layout: [P, NZK*P] zero strip viewed as [P, NZK, P]; the data block
    # is k=DATA_K (columns [DATA_K*P, DATA_K*P + P)).  The lhsT view for
    # tile t (batch b, data column t) is pzv[:, DATA_K-b : DATA_K-b+BL, t]:
    # position k==b lands on the data column, every other k on zeros.
    assert nt == P, "pz addressing assumes one data k-block"
    NZK = 2 * BL - 1
    DATA_K = BL - 1

    nc = bacc.Bacc("TRN2")
    emb = nc.dram_tensor("emb", [n * BL, ED], dt, kind="ExternalInput")
    web = nc.dram_tensor("web", [P, ED], dt, kind="ExternalInput")
    outd = nc.dram_tensor("out", [BL, ED], f32, kind="ExternalOutput")

    # [n*BL, ED] rows -> [P, nt, ED] view (tile t = rows [128t, 128t+128)).
    emb_r = emb[:, :].rearrange("(t p) e -> p t e", p=P)

    with tile.TileContext(nc) as tc:
        with (
            tc.tile_pool(name="consts", bufs=1) as consts,
            tc.tile_pool(name="embp", bufs=emb_bufs) as embp,
            tc.tile_pool(name="tmpd", bufs=tmp_bufs) as tmpd,
            tc.tile_pool(name="tmpa", bufs=tmp_bufs) as tmpa,
            tc.tile_pool(name="tmpg", bufs=tmp_bufs) as tmpg,
            tc.tile_pool(name="lgpD", bufs=len(groups)) as lgpD,
            tc.tile_pool(name="lgpA", bufs=len(groups)) as lgpA,
            tc.tile_pool(name="lgpG", bufs=len(groups)) as lgpG,
            tc.tile_pool(name="smalls", bufs=2) as smalls,
            tc.tile_pool(name="psum", bufs=2, space="PSUM") as psum,
        ):
            web_s = consts.tile([P, ED], dt)
            nc.sync.dma_start(out=web_s, in_=web[:, :])
            ones_s = consts.tile([P, 1], f32)
            nc.vector.memset(ones_s[:, :], 1.0)

            pz = consts.tile([P, NZK * P], dt)  # zero strip + exp-weights
            nc.vector.memset(pz[:, :], 0.0)

            # Prime the const-tile dependencies on each consuming engine so
            # the hot-loop instructions need at most one sync wait each (the
            # 2-src-2-dst STT encoding only has a single wait slot).
            dummy_v = smalls.tile([P, 1], dt)
            nc.vector.tensor_copy(out=dummy_v, in_=web_s[:, 0:1])
            if any(sp[2] for sp in splits):
                dummy_g = smalls.tile([P, 1], dt)
                nc.gpsimd.tensor_copy(out=dummy_g, in_=web_s[:, 0:1])

            rep_ctx = (
                tc.For_i(0, reps, 1, hint_engines=(mybir.EngineType.PE,))
                if reps > 1
                else nullcontext()
            )
            with rep_ctx:
                _kernel_body(
                    nc, tc, mybir, dt, f32, groups, splits, nt,
                    emb_r, web_s, ones_s, pz, NZK, DATA_K,
                    embp, tmpd, tmpa, tmpg, lgpD, lgpA, lgpG,
                    smalls, psum, outd,
                )

    nc.finalize()
    return nc


def _kernel_body(
    nc, tc, mybir, dt, f32, groups, splits, nt,
    emb_r, web_s, ones_s, pz, NZK, DATA_K,
    embp, tmpd, tmpa, tmpg, lgpD, lgpA, lgpG,
    smalls, psum, outd,
):
    pzv = pz[:, :].rearrange("p (k c) -> p k c", k=NZK)
    pdata = pzv[:, DATA_K, :]  # [P, nt] exp-weight columns, col t = tile t
    # Double-buffered per-iteration PSUM accumulator: in reps mode the next
    # iteration's matmul chain starts in the other bank while the tail of
    # this one is still being read.
    acc = psum.tile([BL, ED], f32)

    def exp_to(lg, c0, c1):
        nc.scalar.activation(
            out=pdata[:, c0:c1],
            in_=lg,
            func=mybir.ActivationFunctionType.Exp,
            bias=0.0,
            scale=1.0,
        )

    t0 = 0
    for gi, s in enumerate(groups):
        d, a, g = splits[gi]
        gt = embp.tile([P, max(groups), ED], dt)
        nc.sync.dma_start(out=gt[:, :s, :], in_=emb_r[:, t0 : t0 + s, :])

        # G lane first in program order so GpSimd starts as soon as the
        # DMA lands; its exp comes last on ACT.
        gtmps = []
        if g:
            lgG = lgpG.tile([P, g], f32, name="lgG", tag="lgG")
            for j in range(d + a, s):
                tmp = tmpg.tile([P, ED], dt, name="tmpG", tag="tmpG")
                nc.gpsimd.tensor_mul(out=tmp, in0=gt[:, j, :], in1=web_s)
                gtmps.append(tmp)
        if d:
            lgD = lgpD.tile([P, d], f32, name="lgD", tag="lgD")
            for j in range(d):
                tmp = tmpd.tile([P, ED], dt, name="tmpD", tag="tmpD")
                nc.vector.scalar_tensor_tensor(
                    out=tmp,
                    in0=gt[:, j, :],
                    scalar=1.0,
                    in1=web_s,
                    op0=mybir.AluOpType.mult,
                    op1=mybir.AluOpType.mult,
                    accum_out=lgD[:, j : j + 1],
                )
            exp_to(lgD, t0, t0 + d)
        if a:
            lgA = lgpA.tile([P, a], f32, name="lgA", tag="lgA")
            tmps = []
            for j in range(d, d + a):
                tmp = tmpa.tile([P, ED], dt, name="tmpA", tag="tmpA")
                nc.vector.tensor_mul(out=tmp, in0=gt[:, j, :], in1=web_s)
                tmps.append(tmp)
            for j, tmp in zip(range(d, d + a), tmps):
                nc.scalar.activation(
                    out=tmp,
                    in_=tmp,
                    func=mybir.ActivationFunctionType.Copy,
                    bias=0.0,
                    scale=1.0,
                    accum_out=lgA[:, j - d : j - d + 1],
                )
            exp_to(lgA, t0 + d, t0 + d + a)
        if g:
            # 4x-mode DVE accumulate of the GpSimd products.
            for jj, tmp in enumerate(gtmps):
                nc.vector.tensor_scalar(
                    out=tmp,
                    in0=tmp,
                    scalar1=1.0,
                    scalar2=0.0,
                    op0=mybir.AluOpType.mult,
                    op1=mybir.AluOpType.add,
                    accum_out=lgG[:, jj : jj + 1],
                )
            exp_to(lgG, t0 + d + a, t0 + s)

        for j in range(s):
            t = t0 + j
            b = t // TPB
            nc.tensor.matmul(
                acc,
                pzv[:, DATA_K - b : DATA_K - b + BL, t : t + 1],
                gt[:, j, :],
                start=(t == 0),
                stop=(t == nt - 1),
            )
        t0 += s

    # Z[b] = sum_n p: free-axis reduce per b-block, then one tiny matmul
    # against a ones-column to fold the partition axis -> [BL, 1].
    red8 = smalls.tile([P, BL], f32)
    pview = pdata.rearrange("p (b t) -> p b t", b=BL)
    nc.vector.tensor_reduce(
        out=red8,
        in_=pview,
        axis=mybir.AxisListType.X,
        op=mybir.AluOpType.add,
    )
    zp = psum.tile([BL, 1], f32)
    nc.tensor.matmul(zp, red8, ones_s, start=True, stop=True)
    rz = smalls.tile([BL, 1], f32)
    nc.vector.reciprocal(out=rz, in_=zp)
    outs = smalls.tile([BL, ED], f32)
    nc.vector.tensor_scalar_mul(out=outs, in0=acc, scalar1=rz)
    nc.sync.dma_start(out=outd[:, :], in_=outs)


def _get_nc(dt_name: str):
    if dt_name not in _cache:
        cfg = dict(_BUILD_CFG.get(dt_name, {}))
        _cache[dt_name] = _build(dt_name, **cfg)
    return _cache[dt_name]


def _make_in_maps(inputs):
    """Shard the full inputs into the 8 per-core input maps.

    state_tm1 / b are unused: the state score is constant over the softmax
    axis and cancels.  The per-core shard is laid out b-major (row = b*N+n)
    so each 128-row tile belongs to a single batch entry.
    """
    emb = np.asarray(inputs["embeddings"], dtype=np.float32)
    Wf = np.asarray(inputs["W"], dtype=np.float32)

    dt_name = COMPUTE_DTYPE
    if dt_name == "float32":
        np_dt = np.float32
    elif dt_name == "float16":
        np_dt = np.float16
    else:
        import ml_dtypes

        np_dt = ml_dtypes.bfloat16

    W_e = Wf[SD:, 0]
    web = np.ascontiguousarray(np.broadcast_to(W_e[None, :], (P, ED))).astype(np_dt)

    in_maps = []
    for c in range(NCORES):
        shard = (
            emb[:, c * BL : (c + 1) * BL, :]
            .transpose(1, 0, 2)
            .reshape(N * BL, ED)
            .astype(np_dt)
        )
        shard = np.ascontiguousarray(shard)
        in_maps.append({"emb": shard, "web": web})
    return in_maps


def kernel(state_tm1, embeddings, W, b):
    global last_result
    from concourse.bass_utils import run_bass_kernel_spmd

    in_maps = _make_in_maps(
        dict(state_tm1=state_tm1, embeddings=embeddings, W=W, b=b)
    )
    nc = _get_nc(COMPUTE_DTYPE)
    res = run_bass_kernel_spmd(nc, in_maps, core_ids=list(range(NCORES)))
    last_result = res
    out = np.concatenate([r["out"] for r in res.results], axis=0)
    return out


# revision 42
# speedup vs baseline: 1.0687x; 1.0687x over previous
"""Trainium2 Bass kernel for the attention-mechanism problem.

Math (reference):
    W_s, W_e = W[:SD], W[SD:]
    score[b]    = state_tm1[b] @ W_s + b_bias                 # [B]
    logits[n,b] = emb[n,b] @ W_e + score[b]                   # [N,B]
    alpha       = softmax(logits, axis=0)                     # over N
    out[b,e]    = sum_n alpha[n,b] * emb[n,b,e]               # [B,ED]

score[b] is constant over the softmax axis N, so it cancels: the kernel
never touches state_tm1 / W_s / b_bias at all.

Strategy: data-parallel over B across 8 cores (B_local = 8 per core).
Per-core shard is laid out b-major on the host: row r = b*N + n, so each
[128, ED] SBUF tile holds 128 consecutive n for ONE b.  Single pass:
    l[r]  = emb[r] . W_e          (rowdot -> per-partition scalar)
    p     = exp(l)                (ScalarE, batched per DMA group)
    acc[b]+= p_tile.T @ tile      (TensorE, PSUM accumulation)
    Z[b]  = sum p                 (free-axis reduce + tiny matmul)
    out   = acc / Z

PE constraint: PSUM matmul outputs must sit at base partition 0, so the
weighted sum needs an [128, BL] lhsT whose only nonzero column is b(t).
Rather than building it per tile (the old mask*p ScalarE op), the exp
writes p STRIDED into column b of a persistent zero-initialized
[128, nt, BL] strip; lhsT for tile t is then the dense contiguous
[128, BL] slice strip[:, t, :] — no per-tile construction op, and a
fast contiguous Ldweights (strided weight APs load slowly on PE).

The rowdot is the dominant elementwise cost (512 MACs/row).  Within
EVERY group the tiles are split across three engine lanes (keeping each
lane's progress locked to the DMA stream, so no lane can fall behind
and stall the in-order PE queue):
    D: VectorE scalar_tensor_tensor w/ accum  (1x mode, ~593ns/tile)
    A: VectorE tensor_mul (2x, ~327ns) + ScalarE Copy w/ accum_out
    G: GpSimd tensor_mul (~1.1us, software path; the backend rejects
       STT/accum on Pool) + VectorE tensor_scalar accum (4x, ~253ns)
Each lane gets its own lg tile (single writer) and its own exp call.
With the per-tile mask-multiply gone and the rowdot spread out, every
compute engine sits below the ~45-50us HBM streaming floor for the
16MB fp16 shard, making the kernel DMA-bound.
"""

import os

import numpy as np

N, B, SD, ED = 2048, 64, 512, 512
NCORES = 8
BL = B // NCORES  # 8 batch entries per core
P = 128  # SBUF partitions
NT = (N * BL) // P  # 128 tiles of [128, ED] per core
TPB = N // P  # 16 tiles per batch entry

# Storage/compute precision for the embeddings (the 256MB streaming input):
#   "float16"  (default): half the HBM traffic, rel err ~3e-4 vs the reference
#   "bfloat16": like float16 but rel err ~2e-3
#   "float32":  exact-ish, but 2x the DMA traffic and 4x slower matmul
COMPUTE_DTYPE = os.environ.get("ATTN_KERNEL_DTYPE", "float16")

# Best-known build configuration per dtype.
_BUILD_CFG = {
    "float16": dict(grp=8, emb_bufs=12, taper=True, fd=0.31, fa=0.38),
    "bfloat16": dict(grp=8, emb_bufs=12, taper=True, fd=0.31, fa=0.38),
    "float32": dict(grp=4, emb_bufs=10, taper=True, fd=0.31, fa=0.38),
}

_cache: dict = {}
last_result = None  # BassKernelResults of the most recent run (for profiling)


def _group_sizes(nt: int, grp: int, taper: bool):
    if taper:
        head = [1, 1, 2, 4]
        tail = [4, 2, 1, 1]
        mid_total = nt - sum(head) - sum(tail)
        assert mid_total % grp == 0
        groups = head + [grp] * (mid_total // grp) + tail
    else:
        assert nt % grp == 0
        groups = [grp] * (nt // grp)
    assert sum(groups) == nt
    return groups


def _lane_splits(groups, fd: float, fa: float):
    """Per-group (d, a, g) tile counts hitting the global fd/fa fractions via
    stateful rounding.  Tiny groups go entirely to the D lane (single-op
    chain -> fastest pipeline start/drain)."""
    splits = []
    done = used_d = used_a = 0
    for s in groups:
        done += s
        if s <= 2:
            d, a = s, 0
        else:
            d = max(0, int(round(fd * done)) - used_d)
            a = max(0, int(round((fd + fa) * done)) - used_a - d - used_d + used_d)
            a = max(0, int(round((fd + fa) * done)) - used_d - used_a - d)
            d = min(d, s)
            a = min(a, s - d)
        used_d += d
        used_a += a
        splits.append((d, a, s - d - a))
    return splits


def _build(
    dt_name: str,
    n: int = N,
    grp: int = 8,
    reps: int = 1,
    emb_bufs: int = 12,
    tmp_bufs: int = 3,
    taper: bool = True,
    fd: float = 0.31,
    fa: float = 0.31,
    mode: str = "full",  # "full" | "dma" (stream-only microbench)
):
    """reps>1 wraps the whole kernel in a device-side For_i loop — used only
    for timing (one RPC amortizes `reps` kernel executions)."""
    import concourse.mybir as mybir
    import concourse.tile as tile
    from concourse import bacc
    from contextlib import nullcontext

    f32 = mybir.dt.float32
    dt = getattr(mybir.dt, dt_name)
    nt = (n * BL) // P
    groups = _group_sizes(nt, grp, taper)
    splits = _lane_splits(groups, fd, fa)

    nc = bacc.Bacc("TRN2")
    emb = nc.dram_tensor("emb", [n * BL, ED], dt, kind="ExternalInput")
    web = nc.dram_tensor("web", [P, ED], dt, kind="ExternalInput")
    outd = nc.dram_tensor("out", [BL, ED], f32, kind="ExternalOutput")

    # Per-group DRAM view: partition p takes the group's s CONSECUTIVE rows
    # [r0 + p*s, r0 + (p+1)*s) -> the DMA source is one contiguous s-KB run
    # per partition (8x bigger descriptors than tile-major order).  Slice i
    # then holds rows {r0 + p*s + i}: still one row per partition, same b
    # across the group, and row order within a tile is irrelevant to the
    # rowdot / weighted sum / Z.
    def group_view(r0, s):
        return emb[r0 : r0 + P * s, :].rearrange("(p i) e -> p i e", i=s)

    with tile.TileContext(nc) as tc:
        with (
            tc.tile_pool(name="consts", bufs=1) as consts,
            tc.tile_pool(name="embp", bufs=emb_bufs) as embp,
            tc.tile_pool(name="tmpd", bufs=tmp_bufs) as tmpd,
            tc.tile_pool(name="tmpa", bufs=tmp_bufs) as tmpa,
            tc.tile_pool(name="tmpg", bufs=tmp_bufs) as tmpg,
            tc.tile_pool(name="lgpD", bufs=len(groups)) as lgpD,
            tc.tile_pool(name="lgpA", bufs=len(groups)) as lgpA,
            tc.tile_pool(name="lgpG", bufs=len(groups)) as lgpG,
            tc.tile_pool(name="smalls", bufs=2) as smalls,
            tc.tile_pool(name="psum", bufs=2, space="PSUM") as psum,
        ):
            web_s = consts.tile([P, ED], dt)
            nc.sync.dma_start(out=web_s, in_=web[:, :])
            ones_s = consts.tile([P, 1], f32)
            nc.vector.memset(ones_s[:, :], 1.0)

            # Persistent lhsT strip: strip[:, t, :] is tile t's dense [P, BL]
            # weights; only column b(t) is ever written (by exp), the rest
            # stay zero from this one-time memset.
            lhstrip = consts.tile([P, nt, BL], dt)
            nc.vector.memset(lhstrip[:, :, :], 0.0)

            # Prime the const-tile dependencies on each consuming engine so
            # the hot-loop instructions need at most one sync wait each (the
            # 2-src-2-dst STT encoding only has a single wait slot).
            dummy_v = smalls.tile([P, 1], dt)
            nc.vector.tensor_copy(out=dummy_v, in_=web_s[:, 0:1])
            if any(sp[2] for sp in splits):
                dummy_g = smalls.tile([P, 1], dt)
                nc.gpsimd.tensor_copy(out=dummy_g, in_=web_s[:, 0:1])

            rep_ctx = (
                tc.For_i(0, reps, 1, hint_engines=(mybir.EngineType.PE,))
                if reps > 1
                else nullcontext()
            )
            with rep_ctx:
                if mode == "dma":
                    _dma_only_body(
                        nc, mybir, dt, f32, groups, group_view, embp, smalls,
                        outd,
                    )
                else:
                    _kernel_body(
                        nc, tc, mybir, dt, f32, groups, splits, nt,
                        group_view, web_s, ones_s, lhstrip,
                        embp, tmpd, tmpa, tmpg, lgpD, lgpA, lgpG,
                        smalls, psum, outd,
                    )

    nc.finalize()
    return nc


def _dma_only_body(nc, mybir, dt, f32, groups, group_view, embp, smalls, outd):
    """Microbench: stream all groups, one trivial DVE reader per group."""
    t0 = 0
    chk = smalls.tile([128, 1], dt)
    for s in groups:
        gt = embp.tile([P, max(groups), ED], dt)
        nc.sync.dma_start(out=gt[:, :s, :], in_=group_view(t0 * P, s))
        nc.vector.tensor_copy(out=chk, in_=gt[:, 0, 0:1])
        t0 += s
    outs = smalls.tile([BL, ED], f32)
    nc.vector.memset(outs[:, :], 0.0)
    nc.sync.dma_start(out=outd[:, :], in_=outs)


def _kernel_body(
    nc, tc, mybir, dt, f32, groups, splits, nt,
    group_view, web_s, ones_s, lhstrip,
    embp, tmpd, tmpa, tmpg, lgpD, lgpA, lgpG,
    smalls, psum, outd,
):
    # Double-buffered per-iteration PSUM accumulator: in reps mode the next
    # iteration's matmul chain starts in the other bank while the tail of
    # this one is still being read.
    acc = psum.tile([BL, ED], f32)

    def exp_to(lg, c0, c1, b):
        # Write p strided into column b of the lhsT strip (stride BL).
        nc.scalar.activation(
            out=lhstrip[:, c0:c1, b],
            in_=lg,
            func=mybir.ActivationFunctionType.Exp,
            bias=0.0,
            scale=1.0,
        )

    t0 = 0
    for gi, s in enumerate(groups):
        d, a, g = splits[gi]
        b = t0 // TPB  # whole group lies in one batch entry's tile block
        gt = embp.tile([P, max(groups), ED], dt)
        nc.sync.dma_start(out=gt[:, :s, :], in_=group_view(t0 * P, s))

        # G lane first in program order so GpSimd starts as soon as the
        # DMA lands; its exp comes last on ACT.
        gtmps = []
        if g:
            lgG = lgpG.tile([P, g], f32, name="lgG", tag="lgG")
            for j in range(d + a, s):
                tmp = tmpg.tile([P, ED], dt, name="tmpG", tag="tmpG")
                nc.gpsimd.tensor_mul(out=tmp, in0=gt[:, j, :], in1=web_s)
                gtmps.append(tmp)
        if d:
            lgD = lgpD.tile([P, d], f32, name="lgD", tag="lgD")
            for j in range(d):
                tmp = tmpd.tile([P, ED], dt, name="tmpD", tag="tmpD")
                nc.vector.scalar_tensor_tensor(
                    out=tmp,
                    in0=gt[:, j, :],
                    scalar=1.0,
                    in1=web_s,
                    op0=mybir.AluOpType.mult,
                    op1=mybir.AluOpType.mult,
                    accum_out=lgD[:, j : j + 1],
                )
            exp_to(lgD, t0, t0 + d, b)
        if a:
            lgA = lgpA.tile([P, a], f32, name="lgA", tag="lgA")
            tmps = []
            for j in range(d, d + a):
                tmp = tmpa.tile([P, ED], dt, name="tmpA", tag="tmpA")
                nc.vector.tensor_mul(out=tmp, in0=gt[:, j, :], in1=web_s)
                tmps.append(tmp)
            for j, tmp in zip(range(d, d + a), tmps):
                nc.scalar.activation(
                    out=tmp,
                    in_=tmp,
                    func=mybir.ActivationFunctionType.Copy,
                    bias=0.0,
                    scale=1.0,
                    accum_out=lgA[:, j - d : j - d + 1],
                )
            exp_to(lgA, t0 + d, t0 + d + a, b)
        if g:
            # 4x-mode DVE accumulate of the GpSimd products.
            for jj, tmp in enumerate(gtmps):
                nc.vector.tensor_scalar(
                    out=tmp,
                    in0=tmp,
                    scalar1=1.0,
                    scalar2=0.0,
                    op0=mybir.AluOpType.mult,
                    op1=mybir.AluOpType.add,
                    accum_out=lgG[:, jj : jj + 1],
                )
            exp_to(lgG, t0 + d + a, t0 + s, b)

        for j in range(s):
            t = t0 + j
            nc.tensor.matmul(
                acc,
                lhstrip[:, t, :],
                gt[:, j, :],
                start=(t == 0),
                stop=(t == nt - 1),
            )
        t0 += s

    # Z[b] = sum_n p: free-axis reduce per b-block, then one tiny matmul
    # against a ones-column to fold the partition axis -> [BL, 1].
    red8 = smalls.tile([P, BL], f32)
    for b in range(BL):
        nc.vector.tensor_reduce(
            out=red8[:, b : b + 1],
            in_=lhstrip[:, b * TPB : (b + 1) * TPB, b],
            axis=mybir.AxisListType.X,
            op=mybir.AluOpType.add,
        )
    zp = psum.tile([BL, 1], f32)
    nc.tensor.matmul(zp, red8, ones_s, start=True, stop=True)
    rz = smalls.tile([BL, 1], f32)
    nc.vector.reciprocal(out=rz, in_=zp)
    outs = smalls.tile([BL, ED], f32)
    nc.vector.tensor_scalar_mul(out=outs, in0=acc, scalar1=rz)
    nc.sync.dma_start(out=outd[:, :], in_=outs)


def _get_nc(dt_name: str):
    if dt_name not in _cache:
        cfg = dict(_BUILD_CFG.get(dt_name, {}))
        _cache[dt_name] = _build(dt_name, **cfg)
    return _cache[dt_name]


def _make_in_maps(inputs):
    """Shard the full inputs into the 8 per-core input maps.

    state_tm1 / b are unused: the state score is constant over the softmax
    axis and cancels.  The per-core shard is laid out b-major (row = b*N+n)
    so each 128-row tile belongs to a single batch entry.
    """
    emb = np.asarray(inputs["embeddings"], dtype=np.float32)
    Wf = np.asarray(inputs["W"], dtype=np.float32)

    dt_name = COMPUTE_DTYPE
    if dt_name == "float32":
        np_dt = np.float32
    elif dt_name == "float16":
        np_dt = np.float16
    else:
        import ml_dtypes

        np_dt = ml_dtypes.bfloat16

    W_e = Wf[SD:, 0]
    web = np.ascontiguousarray(np.broadcast_to(W_e[None, :], (P, ED))).astype(np_dt)

    in_maps = []
    for c in range(NCORES):
        shard = (
            emb[:, c * BL : (c + 1) * BL, :]
            .transpose(1, 0, 2)
            .reshape(N * BL, ED)
            .astype(np_dt)
        )
        shard = np.ascontiguousarray(shard)
        in_maps.append({"emb": shard, "web": web})
    return in_maps


def kernel(state_tm1, embeddings, W, b):
    global last_result
    from concourse.bass_utils import run_bass_kernel_spmd

    in_maps = _make_in_maps(
        dict(state_tm1=state_tm1, embeddings=embeddings, W=W, b=b)
    )
    nc = _get_nc(COMPUTE_DTYPE)
    res = run_bass_kernel_spmd(nc, in_maps, core_ids=list(range(NCORES)))
    last_result = res
    out = np.concatenate([r["out"] for r in res.results], axis=0)
    return out
